# revision 4
# baseline (speedup 1.0000x reference)
"""Trainium2 Bass kernel for nn_CrossAttnModule (B=8,N1=N2=4096,C=512,P=256,H=8,MLP=2048).

Sharding: data-parallel over B across the 8 NeuronCores (one batch element per
core); all weights replicated. Per core everything runs as fp32r matmuls.
"""
import sys

for _p in ("/opt/trn_rl_repo", "/opt/trn_rl_repo/concourse"):
    if _p not in sys.path:
        sys.path.insert(0, _p)

import numpy as np

B, N1, N2, C, P, H, MLP = 8, 4096, 4096, 512, 256, 8, 2048
HD = C // H  # 64

_CACHE = {}


def _build(temp_vals, use_bias, use_pos):
    import concourse.bass as bass
    import concourse.bacc as bacc
    import concourse.mybir as mybir
    import concourse.tile as tile
    from concourse.masks import make_identity

    dt = mybir.dt
    AFT = mybir.ActivationFunctionType
    f32, f32r = dt.float32, dt.float32r

    nc = bacc.Bacc("TRN2", target_bir_lowering=False, debug=False, num_devices=8)

    # ---- external I/O (per core) ----
    x1T = nc.dram_tensor("x1T", [C, N1], f32, kind="ExternalInput")
    x2T = nc.dram_tensor("x2T", [C, N2], f32, kind="ExternalInput")
    Wq = nc.dram_tensor("Wq", [C, C], f32, kind="ExternalInput")
    Wkv = nc.dram_tensor("Wkv", [C, 2 * C], f32, kind="ExternalInput")
    Wp = nc.dram_tensor("Wp", [N2, P], f32, kind="ExternalInput")
    W1 = nc.dram_tensor("W1", [C, MLP], f32, kind="ExternalInput")
    W2 = nc.dram_tensor("W2", [MLP, C], f32, kind="ExternalInput")
    if use_bias:
        bq_d = nc.dram_tensor("bq", [C], f32, kind="ExternalInput")
        bkv_d = nc.dram_tensor("bkv", [2 * C], f32, kind="ExternalInput")
        bp_d = nc.dram_tensor("bp", [P], f32, kind="ExternalInput")
        b1_d = nc.dram_tensor("b1", [MLP], f32, kind="ExternalInput")
        b2_d = nc.dram_tensor("b2", [C], f32, kind="ExternalInput")
    if use_pos:
        pqT_d = nc.dram_tensor("pqT", [C, N1], f32, kind="ExternalInput")
        pqN_d = nc.dram_tensor("pqN", [N1, C], f32, kind="ExternalInput")
        pkN_d = nc.dram_tensor("pkN", [N2, C], f32, kind="ExternalInput")
    out_d = nc.dram_tensor("out", [N1, C], f32, kind="ExternalOutput")

    # ---- internal DRAM spill ----
    qT_dram = nc.dram_tensor("qT_dram", [C, N1], f32)
    qnat_dram = nc.dram_tensor("qnat_dram", [N1, C], f32)
    xperm_dram = nc.dram_tensor("xperm_dram", [N1, C], f32)
    dden_dram = nc.dram_tensor("dden_dram", [128, 512], f32)
    y_dram = nc.dram_tensor("y_dram", [N1, C], f32)
    yT_dram = nc.dram_tensor("yT_dram", [C, N1], f32)

    with tile.TileContext(nc) as tc:
        glob = tc.alloc_tile_pool(name="glob", bufs=1)
        # cross-phase smalls
        ident = glob.tile([128, 128], f32, tag="ident")
        make_identity(nc, ident)
        kp_sb = [glob.tile([128, 512], f32r, tag=f"kp{j}", name=f"kp_sb{j}") for j in range(2)]
        vpe = [glob.tile([128, 8 * 65], f32r, tag=f"vpe{pc}", name=f"vpe{pc}") for pc in range(2)]
        if use_bias:
            ones1 = glob.tile([1, 128], f32, tag="ones1f")
            nc.vector.memset(ones1, 1.0)
            ones1r = glob.tile([1, 128], f32r, tag="ones1r")
            nc.vector.tensor_copy(ones1r, ones1)
            bq_sb = glob.tile([128, 4], f32, tag="bq")  # [p, co]: bq[co*128+p]
            nc.sync.dma_start(out=bq_sb, in_=bass.AP(tensor=bq_d.ap().tensor, offset=0, ap=[[1, 128], [128, 4]]))
            bqr_sb = glob.tile([1, 512], f32r, tag="bqr")
            nc.sync.dma_start(out=bqr_sb, in_=bq_d.ap().unsqueeze(0).bitcast(f32r))
            bkvr = glob.tile([1, 1024], f32r, tag="bkvr")
            nc.sync.dma_start(out=bkvr, in_=bkv_d.ap().unsqueeze(0).bitcast(f32r))
            bp_sb = glob.tile([128, 2], f32, tag="bp")
            nc.sync.dma_start(out=bp_sb, in_=bass.AP(tensor=bp_d.ap().tensor, offset=0, ap=[[1, 128], [128, 2]]))
            bp2r = glob.tile([1, 512], f32r, tag="bp2r")
            nc.sync.dma_start(out=bp2r[:, 0:256], in_=bp_d.ap().unsqueeze(0).bitcast(f32r))
            nc.sync.dma_start(out=bp2r[:, 256:512], in_=bp_d.ap().unsqueeze(0).bitcast(f32r))
            b1_sb = glob.tile([128, 16], f32, tag="b1")
            nc.sync.dma_start(out=b1_sb, in_=bass.AP(tensor=b1_d.ap().tensor, offset=0, ap=[[1, 128], [128, 16]]))
            b2r = glob.tile([1, 512], f32r, tag="b2r")
            nc.sync.dma_start(out=b2r, in_=b2_d.ap().unsqueeze(0).bitcast(f32r))

        # ================= stage A + B =================
        with tc.tile_pool(name="ab_sb", bufs=1) as ab:
            wq_sb = [ab.tile([128, C], f32r, tag=f"wq{cc}", name=f"wq_sb{cc}") for cc in range(4)]
            for cc in range(4):
                nc.sync.dma_start(out=wq_sb[cc], in_=Wq.ap()[cc * 128:(cc + 1) * 128, :].bitcast(f32r))
            # ---- stage A: qT + qnat ----
            aps_pool = tc.alloc_tile_pool(name="a_ps", bufs=1, space="PSUM")
            for nq in range(8):
                x1t_in = []
                for cc in range(4):
                    t_ = ab.tile([128, 512], f32r, tag=f"x1t{cc}")
                    nc.sync.dma_start(out=t_, in_=x1T.ap()[cc * 128:(cc + 1) * 128, nq * 512:(nq + 1) * 512].bitcast(f32r))
                    x1t_in.append(t_)
                if use_pos:
                    pqt_t = ab.tile([128, 4, 512], f32, tag="pqt")
                    nc.sync.dma_start(out=pqt_t, in_=pqT_d.ap().rearrange("(a p) n -> p a n", p=128)[:, :, nq * 512:(nq + 1) * 512])
                for co in range(4):
                    ps = aps_pool.tile([128, 512], f32, tag="qt_ps", bufs=2)
                    for cc in range(4):
                        nc.tensor.matmul(ps, wq_sb[cc][:, co * 128:(co + 1) * 128], x1t_in[cc],
                                         start=(cc == 0), stop=(cc == 3))
                    st = ab.tile([128, 512], f32, tag="qt_st", bufs=3)
                    if use_bias:
                        nc.scalar.activation(st, ps, AFT.Identity, bias=bq_sb[:, co:co + 1])
                        if use_pos:
                            nc.vector.tensor_add(st, st, pqt_t[:, co, :])
                    elif use_pos:
                        nc.vector.tensor_add(st, ps, pqt_t[:, co, :])
                    else:
                        nc.vector.tensor_copy(st, ps)
                    nc.sync.dma_start(out=qT_dram.ap()[co * 128:(co + 1) * 128, nq * 512:(nq + 1) * 512], in_=st)
                for sub in range(4):
                    i = nq * 4 + sub
                    ps = aps_pool.tile([128, 512], f32, tag="qn_ps", bufs=2)
                    for cc in range(4):
                        nc.tensor.matmul(ps, x1t_in[cc][:, sub * 128:(sub + 1) * 128], wq_sb[cc],
                                         start=(cc == 0), stop=(cc == 3))
                    if use_bias:
                        nc.tensor.matmul(ps, ones1r, bqr_sb, start=False, stop=True, skip_group_check=True)
                    st = ab.tile([128, 512], f32, tag="qn_st", bufs=3)
                    if use_pos:
                        pq_t = ab.tile([128, 512], f32, tag="pqn")
                        nc.sync.dma_start(out=pq_t, in_=pqN_d.ap()[i * 128:(i + 1) * 128, :])
                        nc.vector.tensor_add(st, ps, pq_t)
                    else:
                        nc.vector.tensor_copy(st, ps)
                    nc.sync.dma_start(out=qnat_dram.ap()[i * 128:(i + 1) * 128, :], in_=st)
            aps_pool.release()
            # ---- stage B: k, v, kp, vpT ----
            bps_pool = tc.alloc_tile_pool(name="b_ps", bufs=1, space="PSUM")
            wkv_sb = [ab.tile([128, 2 * C], f32r, tag=f"wkv{cc}", name=f"wkv_sb{cc}") for cc in range(4)]
            for cc in range(4):
                nc.sync.dma_start(out=wkv_sb[cc], in_=Wkv.ap()[cc * 128:(cc + 1) * 128, :].bitcast(f32r))
            kp_ps = [bps_pool.tile([128, 256], f32, tag=f"kp_ps{j}", name=f"kp_ps{j}") for j in range(4)]
            vp_ps = [bps_pool.tile([128, 512], f32, tag=f"vp_ps{pc}", name=f"vp_ps{pc}") for pc in range(2)]
            x2t_in = None
            for n2c in range(32):
                blk, sl = n2c // 4, n2c % 4
                if sl == 0:
                    x2t_in = []
                    for cc in range(4):
                        t_ = ab.tile([128, 512], f32r, tag=f"x2t{cc}", bufs=2)
                        nc.sync.dma_start(out=t_, in_=x2T.ap()[cc * 128:(cc + 1) * 128, blk * 512:(blk + 1) * 512].bitcast(f32r))
                        x2t_in.append(t_)
                wp_in = ab.tile([128, 256], f32r, tag="wp_in", bufs=3)
                nc.sync.dma_start(out=wp_in, in_=Wp.ap()[n2c * 128:(n2c + 1) * 128, :].bitcast(f32r))
                kps = bps_pool.tile([128, 512], f32, tag="k_ps", bufs=1)
                vps = bps_pool.tile([128, 512], f32, tag="v_ps", bufs=1)
                for cc in range(4):
                    nc.tensor.matmul(kps, x2t_in[cc][:, sl * 128:(sl + 1) * 128], wkv_sb[cc][:, 0:512],
                                     start=(cc == 0), stop=(cc == 3 and not use_bias))
                if use_bias:
                    nc.tensor.matmul(kps, ones1r, bkvr[:, 0:512], start=False, stop=True, skip_group_check=True)
                for cc in range(4):
                    nc.tensor.matmul(vps, x2t_in[cc][:, sl * 128:(sl + 1) * 128], wkv_sb[cc][:, 512:1024],
                                     start=(cc == 0), stop=(cc == 3 and not use_bias))
                if use_bias:
                    nc.tensor.matmul(vps, ones1r, bkvr[:, 512:1024], start=False, stop=True, skip_group_check=True)
                k_sb = ab.tile([128, 512], f32r, tag="k_sb", bufs=3)
                v_sb = ab.tile([128, 512], f32r, tag="v_sb", bufs=3)
                if use_pos:
                    pk_t = ab.tile([128, 512], f32, tag="pkn", bufs=2)
                    nc.sync.dma_start(out=pk_t, in_=pkN_d.ap()[n2c * 128:(n2c + 1) * 128, :])
                    nc.vector.tensor_add(k_sb, kps, pk_t)
                else:
                    nc.vector.tensor_copy(k_sb, kps)
                nc.vector.tensor_copy(v_sb, vps)
                for hp in range(4):
                    nc.tensor.matmul(kp_ps[hp],
                                     k_sb[:, hp * 128:(hp + 1) * 128], wp_in,
                                     start=(n2c == 0), stop=(n2c == 31 and not use_bias))
                for pc in range(2):
                    nc.tensor.matmul(vp_ps[pc], wp_in[:, pc * 128:(pc + 1) * 128], v_sb,
                                     start=(n2c == 0), stop=(n2c == 31))
            if use_bias:
                for hp in range(4):
                    nc.tensor.matmul(kp_ps[hp], ones1r, bp2r[:, 0:256], start=False, stop=True, skip_group_check=True)
            for hp in range(4):
                nc.vector.tensor_copy(kp_sb[hp // 2][:, (hp % 2) * 256:(hp % 2) * 256 + 256], kp_ps[hp])
            for pc in range(2):
                vv = vpe[pc].rearrange("p (h e) -> p h e", e=65)
                if use_bias:
                    nc.scalar.activation(vv[:, :, 0:64], vp_ps[pc].rearrange("p (h e) -> p h e", e=64),
                                         AFT.Identity, bias=bp_sb[:, pc:pc + 1])
                else:
                    nc.vector.tensor_copy(vv[:, :, 0:64], vp_ps[pc].rearrange("p (h e) -> p h e", e=64))
                of = glob.tile([128, 8], f32, tag="onesf")
                nc.vector.memset(of, 1.0)
                nc.vector.tensor_copy(vv[:, :, 64:65], of.unsqueeze(2))
            bps_pool.release()

        # ================= stage C: attention =================
        with tc.tile_pool(name="c_sb", bufs=1) as cp, \
             tc.tile_pool(name="c_ps", bufs=1, space="PSUM") as cps:
            for nq in range(8):
                qt_in = []
                for t in range(4):
                    t_ = cp.tile([128, 512], f32r, tag=f"qt{t}", bufs=2)
                    nc.sync.dma_start(out=t_, in_=qT_dram.ap()[t * 128:(t + 1) * 128, nq * 512:(nq + 1) * 512].bitcast(f32r))
                    qt_in.append(t_)
                for t in range(4):
                    expT = [[None, None], [None, None]]
                    for hh in range(2):
                        for pc in range(2):
                            aps = cps.tile([128, 512], f32, tag="attn_ps", bufs=4)
                            nc.tensor.matmul(aps,
                                             kp_sb[t // 2][hh * 64:(hh + 1) * 64, (t % 2) * 256 + pc * 128:(t % 2) * 256 + (pc + 1) * 128],
                                             qt_in[t][hh * 64:(hh + 1) * 64, :],
                                             start=True, stop=True)
                            e_ = cp.tile([128, 512], f32r, tag="expT", bufs=8)
                            nc.scalar.activation(e_, aps, AFT.Exp, scale=float(temp_vals[2 * t + hh]))
                            expT[hh][pc] = e_
                    for hh in range(2):
                        h = 2 * t + hh
                        xps = cps.tile([65, 512], f32, tag="x_ps", bufs=2)
                        for pc in range(2):
                            nc.tensor.matmul(xps, vpe[pc][:, h * 65:(h + 1) * 65], expT[hh][pc],
                                             start=(pc == 0), stop=(pc == 1))
                        xd = cp.tile([65, 520], f32, tag="xd65", bufs=4)
                        nc.vector.tensor_copy(xd[:, 0:512], xps)
                        # scatter rows d -> xperm rows d*64 + (h*8+nq), cols j
                        nc.sync.dma_start(
                            out=bass.AP(tensor=xperm_dram.ap().tensor, offset=(h * 8 + nq) * 512,
                                        ap=[[64 * 512, 64], [1, 512]]),
                            in_=xd[0:64, 0:512])
                        # denominators -> dden rows dl*64 + h*8 + nq  (dl in {0,1})
                        for dl in range(2):
                            nc.sync.dma_start(
                                out=dden_dram.ap()[dl * 64 + h * 8 + nq:dl * 64 + h * 8 + nq + 1, :],
                                in_=xd[64:65, 0:512])

        # ================= stage D: permute + add&norm + transpose =================
        with tc.tile_pool(name="d_sb", bufs=1) as dp, \
             tc.tile_pool(name="d_ps", bufs=1, space="PSUM") as dps:
            draw = dp.tile([128, 512], f32, tag="draw")
            nc.sync.dma_start(out=draw, in_=dden_dram.ap())
            drec = dp.tile([128, 512], f32, tag="drec")
            nc.vector.reciprocal(drec, draw)
            for i in range(32):
                xp = dp.tile([128, 512], f32, tag="xp", bufs=3)
                nc.sync.dma_start(out=xp, in_=xperm_dram.ap()[i * 128:(i + 1) * 128, :])
                qn = dp.tile([128, 512], f32, tag="qn_in", bufs=3)
                nc.sync.dma_start(out=qn, in_=qnat_dram.ap()[i * 128:(i + 1) * 128, :])
                zt = dp.tile([128, 512], f32, tag="zt", bufs=3)
                nc.vector.tensor_mul(zt, xp, drec)
                nc.vector.tensor_add(zt, zt, qn)
                scr = dp.tile([128, 512], f32, tag="scr", bufs=2)
                ss = dp.tile([128, 1], f32, tag="ss", bufs=2)
                nc.scalar.activation(scr, zt, AFT.Square, accum_out=ss)
                lg = dp.tile([128, 1], f32, tag="lg", bufs=2)
                nc.scalar.activation(lg, ss, AFT.Ln)
                rn = dp.tile([128, 1], f32, tag="rn", bufs=2)
                nc.scalar.activation(rn, lg, AFT.Exp, scale=-0.5)
                yt = dp.tile([128, 512], f32, tag="yt", bufs=3)
                nc.vector.tensor_scalar_mul(yt, zt, rn)
                nc.sync.dma_start(out=y_dram.ap()[i * 128:(i + 1) * 128, :], in_=yt)
                for cc in range(4):
                    tp = dps.tile([128, 128], f32, tag="tr_ps", bufs=4)
                    nc.tensor.transpose(tp, yt[:, cc * 128:(cc + 1) * 128], ident)
                    ys = dp.tile([128, 128], f32, tag="yts", bufs=4)
                    nc.vector.tensor_copy(ys, tp)
                    nc.sync.dma_start(out=yT_dram.ap()[cc * 128:(cc + 1) * 128, i * 128:(i + 1) * 128], in_=ys)

        # ================= stage E: FFN =================
        with tc.tile_pool(name="e_sb", bufs=1) as ep, \
             tc.tile_pool(name="e_ps", bufs=1, space="PSUM") as eps:
            w1_sb = [ep.tile([128, MLP], f32r, tag=f"w1{cc}", name=f"w1_sb{cc}") for cc in range(4)]
            for cc in range(4):
                nc.sync.dma_start(out=w1_sb[cc], in_=W1.ap()[cc * 128:(cc + 1) * 128, :].bitcast(f32r))
            w2_sb = [ep.tile([128, C], f32r, tag=f"w2{m}", name=f"w2_sb{m}") for m in range(16)]
            for m in range(16):
                nc.sync.dma_start(out=w2_sb[m], in_=W2.ap()[m * 128:(m + 1) * 128, :].bitcast(f32r))
            for nq in range(8):
                yt_in = []
                for cc in range(4):
                    t_ = ep.tile([128, 512], f32r, tag=f"ytin{cc}", bufs=2)
                    nc.sync.dma_start(out=t_, in_=yT_dram.ap()[cc * 128:(cc + 1) * 128, nq * 512:(nq + 1) * 512].bitcast(f32r))
                    yt_in.append(t_)
                h1t = []
                for m in range(16):
                    ps = eps.tile([128, 512], f32, tag="h1_ps", bufs=2)
                    for cc in range(4):
                        nc.tensor.matmul(ps, w1_sb[cc][:, m * 128:(m + 1) * 128], yt_in[cc],
                                         start=(cc == 0), stop=(cc == 3))
                    ht = ep.tile([128, 512], f32r, tag=f"h1t{m}")
                    if use_bias:
                        nc.scalar.activation(ht, ps, AFT.Gelu, bias=b1_sb[:, m:m + 1])
                    else:
                        nc.scalar.activation(ht, ps, AFT.Gelu)
                    h1t.append(ht)
                for sub in range(4):
                    i = nq * 4 + sub
                    ps = eps.tile([128, 512], f32, tag="h2_ps", bufs=2)
                    for m in range(16):
                        nc.tensor.matmul(ps, h1t[m][:, sub * 128:(sub + 1) * 128], w2_sb[m],
                                         start=(m == 0), stop=(m == 15 and not use_bias))
                    if use_bias:
                        nc.tensor.matmul(ps, ones1r, b2r, start=False, stop=True, skip_group_check=True)
                    yin = ep.tile([128, 512], f32, tag="y_in", bufs=3)
                    nc.sync.dma_start(out=yin, in_=y_dram.ap()[i * 128:(i + 1) * 128, :])
                    z2 = ep.tile([128, 512], f32, tag="z2", bufs=3)
                    nc.vector.tensor_add(z2, ps, yin)
                    scr = ep.tile([128, 512], f32, tag="scr2", bufs=2)
                    ss = ep.tile([128, 1], f32, tag="ss2", bufs=2)
                    nc.scalar.activation(scr, z2, AFT.Square, accum_out=ss)
                    lg = ep.tile([128, 1], f32, tag="lg2", bufs=2)
                    nc.scalar.activation(lg, ss, AFT.Ln)
                    rn = ep.tile([128, 1], f32, tag="rn2", bufs=2)
                    nc.scalar.activation(rn, lg, AFT.Exp, scale=-0.5)
                    ot = ep.tile([128, 512], f32, tag="ot", bufs=3)
                    nc.vector.tensor_scalar_mul(ot, z2, rn)
                    nc.sync.dma_start(out=out_d.ap()[i * 128:(i + 1) * 128, :], in_=ot)
        glob.release()
    nc.compile()
    return nc


def kernel(**inputs):
    from concourse.bass_utils import run_bass_kernel_spmd

    x1 = np.asarray(inputs["x1"], np.float32)
    x2 = np.asarray(inputs["x2"], np.float32)
    temp = np.asarray(inputs["temperature"], np.float32).reshape(H)
    biases = [np.asarray(inputs[k], np.float32) for k in ("bq", "bkv", "bp", "b1", "b2")]
    use_bias = any(np.any(b) for b in biases)
    pos_q = np.asarray(inputs["pos_q"], np.float32).reshape(N1, C)
    pos_k = np.asarray(inputs["pos_k"], np.float32).reshape(N2, C)
    use_pos = bool(np.any(pos_q) or np.any(pos_k))

    key = (tuple(np.round(temp, 7).tolist()), use_bias, use_pos)
    if key not in _CACHE:
        _CACHE[key] = _build(temp, use_bias, use_pos)
    nc = _CACHE[key]

    shared = {
        "Wq": np.ascontiguousarray(inputs["Wq"], dtype=np.float32),
        "Wkv": np.ascontiguousarray(inputs["Wkv"], dtype=np.float32),
        "Wp": np.ascontiguousarray(inputs["Wp"], dtype=np.float32),
        "W1": np.ascontiguousarray(inputs["W1"], dtype=np.float32),
        "W2": np.ascontiguousarray(inputs["W2"], dtype=np.float32),
    }
    if use_bias:
        shared.update(bq=biases[0], bkv=biases[1], bp=biases[2], b1=biases[3], b2=biases[4])
    if use_pos:
        shared.update(pqT=np.ascontiguousarray(pos_q.T), pqN=pos_q, pkN=pos_k)
    in_maps = []
    for b in range(B):
        m = dict(shared)
        m["x1T"] = np.ascontiguousarray(x1[b].T)
        m["x2T"] = np.ascontiguousarray(x2[b].T)
        in_maps.append(m)
    res = run_bass_kernel_spmd(nc, in_maps, core_ids=list(range(B)))
    return np.stack([res.results[b]["out"] for b in range(B)]).astype(np.float32)


# revision 6
# speedup vs baseline: 10851.7216x; 10851.7216x over previous
"""Trainium2 Bass kernel for nn_CrossAttnModule (B=8,N1=N2=4096,C=512,P=256,H=8,MLP=2048).

Sharding: data-parallel over B across the 8 NeuronCores (one batch element per
core); all weights replicated. Per core everything runs as fp32r matmuls.
"""
import sys

for _p in ("/opt/trn_rl_repo", "/opt/trn_rl_repo/concourse"):
    if _p not in sys.path:
        sys.path.insert(0, _p)

import numpy as np

B, N1, N2, C, P, H, MLP = 8, 4096, 4096, 512, 256, 8, 2048
HD = C // H  # 64

_CACHE = {}


def _build(temp_vals, use_bias, use_pos):
    import concourse.bass as bass
    import concourse.bacc as bacc
    import concourse.mybir as mybir
    import concourse.tile as tile
    from concourse.masks import make_identity

    dt = mybir.dt
    AFT = mybir.ActivationFunctionType
    f32, f32r = dt.float32, dt.float32r

    nc = bacc.Bacc("TRN2", target_bir_lowering=False, debug=False, num_devices=8)

    # ---- external I/O (per core) ----
    x1T = nc.dram_tensor("x1T", [C, N1], f32, kind="ExternalInput")
    x2T = nc.dram_tensor("x2T", [C, N2], f32, kind="ExternalInput")
    Wq = nc.dram_tensor("Wq", [C, C], f32, kind="ExternalInput")
    Wkv = nc.dram_tensor("Wkv", [C, 2 * C], f32, kind="ExternalInput")
    Wp = nc.dram_tensor("Wp", [N2, P], f32, kind="ExternalInput")
    W1 = nc.dram_tensor("W1", [C, MLP], f32, kind="ExternalInput")
    W2 = nc.dram_tensor("W2", [MLP, C], f32, kind="ExternalInput")
    if use_bias:
        bq_d = nc.dram_tensor("bq", [C], f32, kind="ExternalInput")
        bkv_d = nc.dram_tensor("bkv", [2 * C], f32, kind="ExternalInput")
        bp_d = nc.dram_tensor("bp", [P], f32, kind="ExternalInput")
        b1_d = nc.dram_tensor("b1", [MLP], f32, kind="ExternalInput")
        b2_d = nc.dram_tensor("b2", [C], f32, kind="ExternalInput")
    if use_pos:
        pqT_d = nc.dram_tensor("pqT", [C, N1], f32, kind="ExternalInput")
        pqN_d = nc.dram_tensor("pqN", [N1, C], f32, kind="ExternalInput")
        pkN_d = nc.dram_tensor("pkN", [N2, C], f32, kind="ExternalInput")
    out_d = nc.dram_tensor("out", [N1, C], f32, kind="ExternalOutput")

    # ---- internal DRAM spill ----
    qnat_dram = nc.dram_tensor("qnat_dram", [N1, C], f32)
    xperm_dram = nc.dram_tensor("xperm_dram", [N1, C], f32)
    dden_dram = nc.dram_tensor("dden_dram", [128, 512], f32)

    with tile.TileContext(nc) as tc:
        glob = tc.alloc_tile_pool(name="glob", bufs=1)
        # cross-phase smalls
        ident = glob.tile([128, 128], f32, tag="ident")
        make_identity(nc, ident)
        kp_sb = [glob.tile([128, 512], f32r, tag=f"kp{j}", name=f"kp_sb{j}") for j in range(2)]
        vpe = [glob.tile([128, 8 * 65], f32r, tag=f"vpe{pc}", name=f"vpe{pc}") for pc in range(2)]
        if use_bias:
            ones1 = glob.tile([1, 128], f32, tag="ones1f")
            nc.vector.memset(ones1, 1.0)
            ones1r = glob.tile([1, 128], f32r, tag="ones1r")
            nc.vector.tensor_copy(ones1r, ones1)
            bq_sb = glob.tile([128, 4], f32, tag="bq")  # [p, co]: bq[co*128+p]
            nc.sync.dma_start(out=bq_sb, in_=bass.AP(tensor=bq_d.ap().tensor, offset=0, ap=[[1, 128], [128, 4]]))
            bqr_sb = glob.tile([1, 512], f32r, tag="bqr")
            nc.sync.dma_start(out=bqr_sb, in_=bq_d.ap().unsqueeze(0).bitcast(f32r))
            bkvr = glob.tile([1, 1024], f32r, tag="bkvr")
            nc.sync.dma_start(out=bkvr, in_=bkv_d.ap().unsqueeze(0).bitcast(f32r))
            bp_sb = glob.tile([128, 2], f32, tag="bp")
            nc.sync.dma_start(out=bp_sb, in_=bass.AP(tensor=bp_d.ap().tensor, offset=0, ap=[[1, 128], [128, 2]]))
            bp2r = glob.tile([1, 512], f32r, tag="bp2r")
            nc.sync.dma_start(out=bp2r[:, 0:256], in_=bp_d.ap().unsqueeze(0).bitcast(f32r))
            nc.sync.dma_start(out=bp2r[:, 256:512], in_=bp_d.ap().unsqueeze(0).bitcast(f32r))
            b1_sb = glob.tile([128, 16], f32, tag="b1")
            nc.sync.dma_start(out=b1_sb, in_=bass.AP(tensor=b1_d.ap().tensor, offset=0, ap=[[1, 128], [128, 16]]))
            b2r = glob.tile([1, 512], f32r, tag="b2r")
            nc.sync.dma_start(out=b2r, in_=b2_d.ap().unsqueeze(0).bitcast(f32r))

        # ================= stage A + B =================
        qt_pool = tc.alloc_tile_pool(name="qt_pool", bufs=1)
        qt_res = [qt_pool.tile([128, N1], f32r, tag=f"qtr{co}", name=f"qt_res{co}") for co in range(4)]
        with tc.tile_pool(name="ab_sb", bufs=1) as ab:
            wq_sb = [ab.tile([128, C], f32r, tag=f"wq{cc}", name=f"wq_sb{cc}") for cc in range(4)]
            for cc in range(4):
                nc.sync.dma_start(out=wq_sb[cc], in_=Wq.ap()[cc * 128:(cc + 1) * 128, :].bitcast(f32r))
            # ---- stage A: qT + qnat ----
            aps_pool = tc.alloc_tile_pool(name="a_ps", bufs=1, space="PSUM")
            for nq in range(8):
                x1t_in = []
                for cc in range(4):
                    t_ = ab.tile([128, 512], f32r, tag=f"x1t{cc}", bufs=2)
                    nc.sync.dma_start(out=t_, in_=x1T.ap()[cc * 128:(cc + 1) * 128, nq * 512:(nq + 1) * 512].bitcast(f32r))
                    x1t_in.append(t_)
                if use_pos:
                    pqt_t = ab.tile([128, 4, 512], f32, tag="pqt")
                    nc.sync.dma_start(out=pqt_t, in_=pqT_d.ap().rearrange("(a p) n -> p a n", p=128)[:, :, nq * 512:(nq + 1) * 512])
                for co in range(4):
                    ps = aps_pool.tile([128, 512], f32, tag="qt_ps", bufs=2)
                    for cc in range(4):
                        nc.tensor.matmul(ps, wq_sb[cc][:, co * 128:(co + 1) * 128], x1t_in[cc],
                                         start=(cc == 0), stop=(cc == 3))
                    dst = qt_res[co][:, nq * 512:(nq + 1) * 512]
                    if use_bias:
                        nc.scalar.activation(dst, ps, AFT.Identity, bias=bq_sb[:, co:co + 1])
                        if use_pos:
                            nc.vector.tensor_add(dst, dst, pqt_t[:, co, :])
                    elif use_pos:
                        nc.vector.tensor_add(dst, ps, pqt_t[:, co, :])
                    else:
                        nc.vector.tensor_copy(dst, ps)
                for sub in range(4):
                    i = nq * 4 + sub
                    ps = aps_pool.tile([128, 512], f32, tag="qn_ps", bufs=2)
                    for cc in range(4):
                        nc.tensor.matmul(ps, x1t_in[cc][:, sub * 128:(sub + 1) * 128], wq_sb[cc],
                                         start=(cc == 0), stop=(cc == 3))
                    if use_bias:
                        nc.tensor.matmul(ps, ones1r, bqr_sb, start=False, stop=True, skip_group_check=True)
                    st = ab.tile([128, 512], f32, tag="qn_st", bufs=3)
                    if use_pos:
                        pq_t = ab.tile([128, 512], f32, tag="pqn")
                        nc.sync.dma_start(out=pq_t, in_=pqN_d.ap()[i * 128:(i + 1) * 128, :])
                        nc.vector.tensor_add(st, ps, pq_t)
                    else:
                        nc.vector.tensor_copy(st, ps)
                    nc.sync.dma_start(out=qnat_dram.ap()[i * 128:(i + 1) * 128, :], in_=st)
            aps_pool.release()
            # ---- stage B: k, v, kp, vpT ----
            bps_pool = tc.alloc_tile_pool(name="b_ps", bufs=1, space="PSUM")
            wkv_sb = [ab.tile([128, 2 * C], f32r, tag=f"wkv{cc}", name=f"wkv_sb{cc}") for cc in range(4)]
            for cc in range(4):
                nc.sync.dma_start(out=wkv_sb[cc], in_=Wkv.ap()[cc * 128:(cc + 1) * 128, :].bitcast(f32r))
            kp_ps = [bps_pool.tile([128, 256], f32, tag=f"kp_ps{j}", name=f"kp_ps{j}") for j in range(4)]
            vp_ps = [bps_pool.tile([128, 512], f32, tag=f"vp_ps{pc}", name=f"vp_ps{pc}") for pc in range(2)]
            x2t_in = None
            for n2c in range(32):
                blk, sl = n2c // 4, n2c % 4
                if sl == 0:
                    x2t_in = []
                    for cc in range(4):
                        t_ = ab.tile([128, 512], f32r, tag=f"x2t{cc}", bufs=2)
                        nc.sync.dma_start(out=t_, in_=x2T.ap()[cc * 128:(cc + 1) * 128, blk * 512:(blk + 1) * 512].bitcast(f32r))
                        x2t_in.append(t_)
                wp_in = ab.tile([128, 256], f32r, tag="wp_in", bufs=3)
                nc.sync.dma_start(out=wp_in, in_=Wp.ap()[n2c * 128:(n2c + 1) * 128, :].bitcast(f32r))
                kps = bps_pool.tile([128, 512], f32, tag="k_ps", bufs=1)
                vps = bps_pool.tile([128, 512], f32, tag="v_ps", bufs=1)
                for cc in range(4):
                    nc.tensor.matmul(kps, x2t_in[cc][:, sl * 128:(sl + 1) * 128], wkv_sb[cc][:, 0:512],
                                     start=(cc == 0), stop=(cc == 3 and not use_bias))
                if use_bias:
                    nc.tensor.matmul(kps, ones1r, bkvr[:, 0:512], start=False, stop=True, skip_group_check=True)
                for cc in range(4):
                    nc.tensor.matmul(vps, x2t_in[cc][:, sl * 128:(sl + 1) * 128], wkv_sb[cc][:, 512:1024],
                                     start=(cc == 0), stop=(cc == 3 and not use_bias))
                if use_bias:
                    nc.tensor.matmul(vps, ones1r, bkvr[:, 512:1024], start=False, stop=True, skip_group_check=True)
                k_sb = ab.tile([128, 512], f32r, tag="k_sb", bufs=3)
                v_sb = ab.tile([128, 512], f32r, tag="v_sb", bufs=3)
                if use_pos:
                    pk_t = ab.tile([128, 512], f32, tag="pkn", bufs=2)
                    nc.sync.dma_start(out=pk_t, in_=pkN_d.ap()[n2c * 128:(n2c + 1) * 128, :])
                    nc.vector.tensor_add(k_sb, kps, pk_t)
                else:
                    nc.vector.tensor_copy(k_sb, kps)
                nc.vector.tensor_copy(v_sb, vps)
                for hp in range(4):
                    nc.tensor.matmul(kp_ps[hp],
                                     k_sb[:, hp * 128:(hp + 1) * 128], wp_in,
                                     start=(n2c == 0), stop=(n2c == 31 and not use_bias))
                for pc in range(2):
                    nc.tensor.matmul(vp_ps[pc], wp_in[:, pc * 128:(pc + 1) * 128], v_sb,
                                     start=(n2c == 0), stop=(n2c == 31))
            if use_bias:
                for hp in range(4):
                    nc.tensor.matmul(kp_ps[hp], ones1r, bp2r[:, 0:256], start=False, stop=True, skip_group_check=True)
            for hp in range(4):
                nc.vector.tensor_copy(kp_sb[hp // 2][:, (hp % 2) * 256:(hp % 2) * 256 + 256], kp_ps[hp])
            for pc in range(2):
                vv = vpe[pc].rearrange("p (h e) -> p h e", e=65)
                if use_bias:
                    nc.scalar.activation(vv[:, :, 0:64], vp_ps[pc].rearrange("p (h e) -> p h e", e=64),
                                         AFT.Identity, bias=bp_sb[:, pc:pc + 1])
                else:
                    nc.vector.tensor_copy(vv[:, :, 0:64], vp_ps[pc].rearrange("p (h e) -> p h e", e=64))
                of = glob.tile([128, 8], f32, tag="onesf")
                nc.vector.memset(of, 1.0)
                nc.vector.tensor_copy(vv[:, :, 64:65], of.unsqueeze(2))
            bps_pool.release()

        # ================= stage C: attention =================
        with tc.tile_pool(name="c_sb", bufs=1) as cp, \
             tc.tile_pool(name="c_ps", bufs=1, space="PSUM") as cps:
            for nq in range(8):
                for t in range(4):
                    abuf = cp.tile([128, 4, 512], f32, tag="abuf", bufs=2)
                    same_temp = float(temp_vals[2 * t]) == float(temp_vals[2 * t + 1])
                    for hh in range(2):
                        for pc in range(2):
                            aps = cps.tile([128, 512], f32, tag="attn_ps", bufs=4)
                            nc.tensor.matmul(aps,
                                             kp_sb[t // 2][hh * 64:(hh + 1) * 64, (t % 2) * 256 + pc * 128:(t % 2) * 256 + (pc + 1) * 128],
                                             qt_res[t][hh * 64:(hh + 1) * 64, nq * 512:(nq + 1) * 512],
                                             start=True, stop=True)
                            nc.vector.tensor_copy(abuf[:, hh * 2 + pc, :], aps)
                    ebuf = cp.tile([128, 4, 512], f32r, tag="expT", bufs=4)
                    if same_temp:
                        nc.scalar.activation(ebuf, abuf, AFT.Exp, scale=float(temp_vals[2 * t]))
                    else:
                        for hh in range(2):
                            nc.scalar.activation(ebuf[:, hh * 2:hh * 2 + 2, :], abuf[:, hh * 2:hh * 2 + 2, :],
                                                 AFT.Exp, scale=float(temp_vals[2 * t + hh]))
                    for hh in range(2):
                        h = 2 * t + hh
                        xps = cps.tile([65, 512], f32, tag="x_ps", bufs=2)
                        for pc in range(2):
                            nc.tensor.matmul(xps, vpe[pc][:, h * 65:(h + 1) * 65], ebuf[:, hh * 2 + pc, :],
                                             start=(pc == 0), stop=(pc == 1))
                        xd = cp.tile([65, 520], f32, tag="xd65", bufs=4)
                        nc.vector.tensor_copy(xd[:, 0:512], xps)
                        # scatter rows d -> xperm rows d*64 + (h*8+nq), cols j
                        nc.sync.dma_start(
                            out=bass.AP(tensor=xperm_dram.ap().tensor, offset=(h * 8 + nq) * 512,
                                        ap=[[64 * 512, 64], [1, 512]]),
                            in_=xd[0:64, 0:512])
                        # denominators -> dden rows dl*64 + h*8 + nq  (dl in {0,1})
                        for dl in range(2):
                            nc.sync.dma_start(
                                out=dden_dram.ap()[dl * 64 + h * 8 + nq:dl * 64 + h * 8 + nq + 1, :],
                                in_=xd[64:65, 0:512])

        qt_pool.release()

        # ============ stages D+E merged: permute+norm interleaved with FFN ============
        with tc.tile_pool(name="de_sb", bufs=1) as dp, \
             tc.tile_pool(name="de_ps", bufs=1, space="PSUM") as dps:
            w1_sb = [dp.tile([128, MLP], f32r, tag=f"w1{cc}", name=f"w1_sb{cc}") for cc in range(4)]
            for cc in range(4):
                nc.sync.dma_start(out=w1_sb[cc], in_=W1.ap()[cc * 128:(cc + 1) * 128, :].bitcast(f32r))
            w2_sb = [dp.tile([128, C], f32r, tag=f"w2{m}", name=f"w2_sb{m}") for m in range(16)]
            for m in range(16):
                nc.sync.dma_start(out=w2_sb[m], in_=W2.ap()[m * 128:(m + 1) * 128, :].bitcast(f32r))
            draw = dp.tile([128, 512], f32, tag="draw")
            nc.sync.dma_start(out=draw, in_=dden_dram.ap())
            drec = dp.tile([128, 512], f32, tag="drec")
            nc.vector.reciprocal(drec, draw)
            for nq in range(8):
                ytin = []
                for cc in range(4):
                    t_ = dp.tile([128, 512], f32r, tag=f"ytin{cc}", bufs=2)
                    ytin.append(t_)
                yts = []
                for sub in range(4):
                    i = nq * 4 + sub
                    # ---- D part: permute + add&norm + transpose ----
                    xp = dp.tile([128, 512], f32, tag="xp", bufs=3)
                    nc.sync.dma_start(out=xp, in_=xperm_dram.ap()[i * 128:(i + 1) * 128, :])
                    qn = dp.tile([128, 512], f32, tag="qn_in", bufs=3)
                    nc.sync.dma_start(out=qn, in_=qnat_dram.ap()[i * 128:(i + 1) * 128, :])
                    zt = dp.tile([128, 512], f32, tag="zt", bufs=3)
                    nc.vector.tensor_mul(zt, xp, drec)
                    nc.vector.tensor_add(zt, zt, qn)
                    scr = dp.tile([128, 512], f32, tag="scr", bufs=2)
                    ss = dp.tile([128, 1], f32, tag="ss", bufs=2)
                    nc.scalar.activation(scr, zt, AFT.Square, accum_out=ss)
                    lg = dp.tile([128, 1], f32, tag="lg", bufs=2)
                    nc.scalar.activation(lg, ss, AFT.Ln)
                    rn = dp.tile([128, 1], f32, tag="rn", bufs=2)
                    nc.scalar.activation(rn, lg, AFT.Exp, scale=-0.5)
                    yt = dp.tile([128, 512], f32, tag="yt", bufs=6)
                    nc.vector.tensor_scalar_mul(yt, zt, rn)
                    yts.append(yt)
                    for cc in range(4):
                        tp = dps.tile([128, 128], f32, tag="tr_ps", bufs=2)
                        nc.tensor.transpose(tp, yt[:, cc * 128:(cc + 1) * 128], ident)
                        nc.vector.tensor_copy(ytin[cc][:, sub * 128:(sub + 1) * 128], tp)
                # ---- E part: FFN for this nq ----
                h1t = []
                for m in range(16):
                    ps = dps.tile([128, 512], f32, tag="h1_ps", bufs=2)
                    for cc in range(4):
                        nc.tensor.matmul(ps, w1_sb[cc][:, m * 128:(m + 1) * 128], ytin[cc],
                                         start=(cc == 0), stop=(cc == 3))
                    ht = dp.tile([128, 512], f32r, tag=f"h1t{m}", name=f"h1t{m}")
                    if use_bias:
                        nc.scalar.activation(ht, ps, AFT.Gelu, bias=b1_sb[:, m:m + 1])
                    else:
                        nc.scalar.activation(ht, ps, AFT.Gelu)
                    h1t.append(ht)
                for sub in range(4):
                    i = nq * 4 + sub
                    ps = dps.tile([128, 512], f32, tag="h2_ps", bufs=2)
                    for m in range(16):
                        nc.tensor.matmul(ps, h1t[m][:, sub * 128:(sub + 1) * 128], w2_sb[m],
                                         start=(m == 0), stop=(m == 15 and not use_bias))
                    if use_bias:
                        nc.tensor.matmul(ps, ones1r, b2r, start=False, stop=True, skip_group_check=True)
                    z2 = dp.tile([128, 512], f32, tag="z2", bufs=3)
                    nc.vector.tensor_add(z2, ps, yts[sub])
                    scr2 = dp.tile([128, 512], f32, tag="scr2", bufs=2)
                    ss2 = dp.tile([128, 1], f32, tag="ss2", bufs=2)
                    nc.scalar.activation(scr2, z2, AFT.Square, accum_out=ss2)
                    lg2 = dp.tile([128, 1], f32, tag="lg2", bufs=2)
                    nc.scalar.activation(lg2, ss2, AFT.Ln)
                    rn2 = dp.tile([128, 1], f32, tag="rn2", bufs=2)
                    nc.scalar.activation(rn2, lg2, AFT.Exp, scale=-0.5)
                    ot = dp.tile([128, 512], f32, tag="ot", bufs=3)
                    nc.vector.tensor_scalar_mul(ot, z2, rn2)
                    nc.sync.dma_start(out=out_d.ap()[i * 128:(i + 1) * 128, :], in_=ot)
        glob.release()
    nc.compile()
    return nc


def kernel(**inputs):
    from concourse.bass_utils import run_bass_kernel_spmd

    x1 = np.asarray(inputs["x1"], np.float32)
    x2 = np.asarray(inputs["x2"], np.float32)
    temp = np.asarray(inputs["temperature"], np.float32).reshape(H)
    biases = [np.asarray(inputs[k], np.float32) for k in ("bq", "bkv", "bp", "b1", "b2")]
    use_bias = any(np.any(b) for b in biases)
    pos_q = np.asarray(inputs["pos_q"], np.float32).reshape(N1, C)
    pos_k = np.asarray(inputs["pos_k"], np.float32).reshape(N2, C)
    use_pos = bool(np.any(pos_q) or np.any(pos_k))

    key = (tuple(np.round(temp, 7).tolist()), use_bias, use_pos)
    if key not in _CACHE:
        _CACHE[key] = _build(temp, use_bias, use_pos)
    nc = _CACHE[key]

    shared = {
        "Wq": np.ascontiguousarray(inputs["Wq"], dtype=np.float32),
        "Wkv": np.ascontiguousarray(inputs["Wkv"], dtype=np.float32),
        "Wp": np.ascontiguousarray(inputs["Wp"], dtype=np.float32),
        "W1": np.ascontiguousarray(inputs["W1"], dtype=np.float32),
        "W2": np.ascontiguousarray(inputs["W2"], dtype=np.float32),
    }
    if use_bias:
        shared.update(bq=biases[0], bkv=biases[1], bp=biases[2], b1=biases[3], b2=biases[4])
    if use_pos:
        shared.update(pqT=np.ascontiguousarray(pos_q.T), pqN=pos_q, pkN=pos_k)
    in_maps = []
    for b in range(B):
        m = dict(shared)
        m["x1T"] = np.ascontiguousarray(x1[b].T)
        m["x2T"] = np.ascontiguousarray(x2[b].T)
        in_maps.append(m)
    res = run_bass_kernel_spmd(nc, in_maps, core_ids=list(range(B)))
    return np.stack([res.results[b]["out"] for b in range(B)]).astype(np.float32)
